# revision 5
# baseline (speedup 1.0000x reference)
"""Trainium2 Bass kernel for nn_Decoder_arch2 (LSTM image-caption decoder).

Reference computation (B=128, T=24 used steps, E=512, H2=1024, V=30000):
  tok = emb[captions]; seq = [pad_emb, tok[:, :23]]           # [B, 24, E]
  x_t = concat(seq_t, features)                               # [B, 2E]
  xg = x @ W_ih.T + b_ih + b_hh                               # [B, 24, 4096]
  24x LSTMCell steps (h = o*tanh(c), c = f*c + i*tanh(g))
  logits_t = h_t @ W_out.T + b_out                            # [B, 24, V]
  out = transpose(logits, (0, 2, 1))                          # [B, V, 24]
(The reference computes 25 steps and drops the last logit column, so step 25
and the last caption token are never needed.)

Sharding: pure data-parallel over batch. 8 cores x 16 batch rows each; every
core holds the full weights and computes its shard end-to-end. No collectives.

Device layouts (per core, partition dim always 128):
  gathered embeddings -> PE-transposed to xT[ec] [128(e), 384(t*16+b)] bf16
  xg_sb  [128, 32gc, 24t, 16b] bf16   (gate g = gc*128 + p; incl. feat+bias)
  hs_sb  [128, 8hc, 24t, 16b] bf16    (hidden u = hc*128 + p)
  W_hh in fp8e4m3 (stationary side only; h stays bf16 — validated 7.9e-3 rel)
  LSTM gates accumulate per-gate in separate PSUM banks (order f,g,i,o) so the
  sigmoid/tanh chain of one gate overlaps the matmul burst of the next; the
  o-gate tail is processed in two hidden-halves so the next step's f-matmuls
  (hc-outer order) start as soon as the first half of h is written.
  projection: W_out tiles stationary, hs chunks moving, out [128(v), 384(t,b)]
  logits staged in bf16; host upcasts to f32 (tolerance 2e-2 >> bf16 eps).

Host pre-transposes/casts all weights (free layout prep) and reassembles the
[128, 30000, 24] output from the per-core [235, 128, 384] tensors.
"""

import sys

if "/opt/trn_rl_repo" not in sys.path:
    sys.path.insert(0, "/opt/trn_rl_repo")

import numpy as np
import ml_dtypes

import concourse.bass as bass
import concourse.bacc as bacc
import concourse.mybir as mybir
import concourse.tile as tile
from concourse.bass_utils import run_bass_kernel_spmd
from concourse.masks import make_identity

bf16 = ml_dtypes.bfloat16
fp8 = ml_dtypes.float8_e4m3
F32 = mybir.dt.float32
BF16 = mybir.dt.bfloat16
FP8E4 = mybir.dt.float8e4
I32 = mybir.dt.int32

B, T, E, V, H2 = 128, 24, 512, 30000, 1024
G = 4 * H2  # 4096
NC_N = 8
BS = B // NC_N  # 16 batch rows per core
NVT = 235  # ceil(30000/128)
VP = NVT * 128  # 30080
NCOL = T * BS  # 384 moving columns (t*16 + b)
STG = 4  # vt tiles per output staging DMA

# torch gate order i,f,g,o -> gc ranges; compute order f,g,i,o so the
# c-update chain (needs f,g,i) finishes under the o matmul burst.
GATE_I, GATE_F, GATE_G, GATE_O = range(4)
# xg gc-production order: i,g,o first so step 0 (which has no f*c term)
# can start its activation chain before the f columns finish.
XG_GC_ORDER = list(range(0, 8)) + list(range(16, 32)) + list(range(8, 16))


def build_nc():
    nc = bacc.Bacc(None, target_bir_lowering=False)

    emb_d = nc.dram_tensor("embB", [V, E], BF16, kind="ExternalInput")
    idx_d = nc.dram_tensor("idx", [128, 3], I32, kind="ExternalInput")
    feat_d = nc.dram_tensor("featT", [128, 4, BS], BF16, kind="ExternalInput")
    wih_d = nc.dram_tensor("wihT", [8, 128, G], BF16, kind="ExternalInput")
    whh_d = nc.dram_tensor("whhT", [8, 128, G], FP8E4, kind="ExternalInput")
    bsum_d = nc.dram_tensor("bsum", [128, 32, BS], F32, kind="ExternalInput")
    bout_d = nc.dram_tensor("bout", [128, NVT], F32, kind="ExternalInput")
    wop_d = nc.dram_tensor("wop", [NVT, 128, H2], BF16, kind="ExternalInput")
    out_d = nc.dram_tensor("out", [NVT, 128, NCOL], BF16, kind="ExternalOutput")

    with tile.TileContext(nc) as tc:
        with (
            tc.tile_pool(name="const", bufs=1) as const,
            tc.tile_pool(name="ge", bufs=3) as gep,
            tc.tile_pool(name="xt", bufs=4) as xtp,
            tc.tile_pool(name="w", bufs=8) as wp,
            tc.tile_pool(name="whhd", bufs=8) as whhdp,
            tc.tile_pool(name="big", bufs=1) as big,
            tc.tile_pool(name="tmp", bufs=2) as tmp,
            tc.tile_pool(name="wout", bufs=8) as woutp,
            tc.tile_pool(name="stage", bufs=3) as stagep,
            tc.tile_pool(name="pa", bufs=2, space="PSUM") as pap,
            tc.tile_pool(name="pg", bufs=4, space="PSUM") as pgp,
            tc.tile_pool(name="po", bufs=2, space="PSUM") as pop,
        ):
            # ---- constants / small inputs ----
            idx_sb = const.tile([128, 3], I32)
            nc.sync.dma_start(idx_sb[:], idx_d[:])
            feat_sb = const.tile([128, 4, BS], BF16)
            nc.sync.dma_start(feat_sb[:], feat_d[:])
            bsum_sb = const.tile([128, 32, BS], F32)
            nc.sync.dma_start(bsum_sb[:], bsum_d[:])
            bout_sb = const.tile([128, NVT], F32)
            nc.sync.dma_start(bout_sb[:], bout_d[:])
            ident = const.tile([128, 128], BF16)
            make_identity(nc, ident)

            # ---- W_ih tiles (feat first: needed for fg right away) ----
            wih_feat = []
            for ec in range(4):
                t_ = wp.tile([128, G], BF16, tag="w")
                nc.sync.dma_start(t_[:], wih_d[4 + ec])
                wih_feat.append(t_)
            # W_hh (fp8) has its own slots; DMA queued before the big seq tiles
            # (2 MB total, so it lands well before the LSTM needs it).
            whh = []
            for hc in range(8):
                t_ = whhdp.tile([128, G], FP8E4, tag="whhd")
                nc.sync.dma_start(t_[:], whh_d[hc])
                whh.append(t_)
            wih_seq = []
            for ec in range(4):
                t_ = wp.tile([128, G], BF16, tag="w")
                nc.sync.dma_start(t_[:], wih_d[ec])
                wih_seq.append(t_)

            # ---- embedding gather + transpose into xT ----
            ge = []
            for r in range(3):
                g_t = gep.tile([128, E], BF16)
                nc.gpsimd.indirect_dma_start(
                    out=g_t[:],
                    out_offset=None,
                    in_=emb_d[:],
                    in_offset=bass.IndirectOffsetOnAxis(ap=idx_sb[:, r : r + 1], axis=0),
                )
                ge.append(g_t)

            xt = [xtp.tile([128, NCOL], BF16, tag="xt", name=f"xt{i}") for i in range(4)]
            for ec in range(4):
                for r in range(3):
                    pt = pap.tile([128, 128], BF16, tag="pa")
                    nc.tensor.transpose(
                        pt[:], ge[r][:, ec * 128 : (ec + 1) * 128], ident[:]
                    )
                    nc.vector.tensor_copy(
                        xt[ec][:, r * 128 : (r + 1) * 128], pt[:]
                    )

            # ---- feature-side gate projection fg = W_ih[:, E:] @ feat + bsum ----
            # ec-outer so matmuls start as soon as the first feat tile lands.
            # NOTE: start=True clears has_written for the WHOLE bank, so with
            # interleaved slice-groups only the very first matmul may set it.
            psum_fg = pap.tile([128, 32, BS], F32, tag="pa")
            for ec in range(4):
                for gc in range(32):
                    nc.tensor.matmul(
                        psum_fg[:, gc, :],
                        wih_feat[ec][:, gc * 128 : (gc + 1) * 128],
                        feat_sb[:, ec, :],
                        start=(ec == 0 and gc == 0),
                        stop=(ec == 3 and gc == 31),
                        skip_group_check=True,
                    )
            fg_sb = big.tile([128, 32, BS], F32, tag="fg")
            nc.vector.tensor_add(fg_sb[:], psum_fg[:], bsum_sb[:])

            # ---- xg GEMM (token side): xg[g, (t,b)] = W_ih[:, :E] @ seq ----
            # gc-major layout; the feat+bias term is fused into the PSUM
            # evacuation (one DVE add per gc, broadcast over t).
            xg_sb = big.tile([128, 32, T, BS], BF16, tag="xg")
            for gc in XG_GC_ORDER:
                psum_xg = pap.tile([128, T, BS], F32, tag="pa")
                for ec in range(4):
                    nc.tensor.matmul(
                        psum_xg[:],
                        wih_seq[ec][:, gc * 128 : (gc + 1) * 128],
                        xt[ec][:],
                        start=(ec == 0),
                        stop=(ec == 3),
                    )
                nc.vector.tensor_add(
                    xg_sb[:, gc],
                    psum_xg[:],
                    fg_sb[:, gc : gc + 1, :].broadcast_to([128, T, BS]),
                )

            # ---- LSTM ----
            hs_sb = big.tile([128, 8, T, BS], BF16, tag="hs")
            c_sb = big.tile([128, 8, BS], F32, tag="c")
            SIG = mybir.ActivationFunctionType.Sigmoid
            TANH = mybir.ActivationFunctionType.Tanh

            def gsl(g, lo=0, hi=8):
                return slice(8 * g + lo, 8 * g + hi)

            for t in range(T):
                if t == 0:
                    # h=0, c=0: gates come straight from xg
                    t_i = tmp.tile([128, 8, BS], F32, tag="ti")
                    t_g = tmp.tile([128, 8, BS], F32, tag="tg")
                    t_o = tmp.tile([128, 8, BS], F32, tag="to")
                    t_c = tmp.tile([128, 8, BS], F32, tag="tc")
                    nc.scalar.activation(t_i[:], xg_sb[:, gsl(GATE_I), 0, :], SIG)
                    nc.scalar.activation(t_g[:], xg_sb[:, gsl(GATE_G), 0, :], TANH)
                    nc.vector.tensor_mul(c_sb[:], t_i[:], t_g[:])
                    nc.scalar.activation(t_c[:], c_sb[:], TANH)
                    for half in range(2):
                        hsl = slice(4 * half, 4 * half + 4)
                        nc.scalar.activation(
                            t_o[:, hsl], xg_sb[:, gsl(GATE_O, 4 * half, 4 * half + 4), 0, :], SIG
                        )
                        nc.vector.tensor_mul(hs_sb[:, hsl, 0, :], t_o[:, hsl], t_c[:, hsl])
                    continue

                h_prev = [hs_sb[:, hc, t - 1, :] for hc in range(8)]

                def gate_mm(g, hc_outer=False):
                    pg_t = pgp.tile([128, 8, BS], F32, tag="pg", name=f"pg{g}_{t}")
                    base = 8 * g
                    if hc_outer:
                        # consume h chunk-by-chunk: first matmuls only need
                        # the first half of h(t-1). One accumulation group for
                        # the whole bank (start clears bank-wide has_written).
                        for hc in range(8):
                            for gcl in range(8):
                                gc = base + gcl
                                nc.tensor.matmul(
                                    pg_t[:, gcl, :],
                                    whh[hc][:, gc * 128 : (gc + 1) * 128],
                                    h_prev[hc],
                                    start=(hc == 0 and gcl == 0),
                                    stop=(hc == 7 and gcl == 7),
                                    skip_group_check=True,
                                )
                    else:
                        for gcl in range(8):
                            gc = base + gcl
                            for hc in range(8):
                                nc.tensor.matmul(
                                    pg_t[:, gcl, :],
                                    whh[hc][:, gc * 128 : (gc + 1) * 128],
                                    h_prev[hc],
                                    start=(hc == 0),
                                    stop=(hc == 7),
                                )
                    return pg_t

                # f first: f*c runs under the g/i bursts
                pg_f = gate_mm(GATE_F, hc_outer=True)
                t_f = tmp.tile([128, 8, BS], F32, tag="tf")
                nc.vector.tensor_add(t_f[:], pg_f[:], xg_sb[:, gsl(GATE_F), t, :])
                nc.scalar.activation(t_f[:], t_f[:], SIG)
                nc.vector.tensor_mul(t_f[:], t_f[:], c_sb[:])

                pg_g = gate_mm(GATE_G)
                t_g = tmp.tile([128, 8, BS], F32, tag="tg")
                nc.vector.tensor_add(t_g[:], pg_g[:], xg_sb[:, gsl(GATE_G), t, :])
                nc.scalar.activation(t_g[:], t_g[:], TANH)

                pg_i = gate_mm(GATE_I)
                t_i = tmp.tile([128, 8, BS], F32, tag="ti")
                nc.vector.tensor_add(t_i[:], pg_i[:], xg_sb[:, gsl(GATE_I), t, :])
                nc.scalar.activation(t_i[:], t_i[:], SIG)
                nc.vector.tensor_mul(t_i[:], t_i[:], t_g[:])
                nc.vector.tensor_add(c_sb[:], t_f[:], t_i[:])
                t_c = tmp.tile([128, 8, BS], F32, tag="tc")
                nc.scalar.activation(t_c[:], c_sb[:], TANH)

                pg_o = gate_mm(GATE_O)
                t_o = tmp.tile([128, 8, BS], F32, tag="to")
                # two hidden-halves: h's first half lands one DVE+ACT+DVE
                # earlier, releasing the next step's f-matmuls (hc-outer)
                for half in range(2):
                    hsl = slice(4 * half, 4 * half + 4)
                    nc.vector.tensor_add(
                        t_o[:, hsl], pg_o[:, hsl],
                        xg_sb[:, gsl(GATE_O, 4 * half, 4 * half + 4), t, :],
                    )
                    nc.scalar.activation(t_o[:, hsl], t_o[:, hsl], SIG)
                    nc.vector.tensor_mul(hs_sb[:, hsl, t, :], t_o[:, hsl], t_c[:, hsl])

            # ---- output projection ----
            stage_t = None
            for vt in range(NVT):
                w_t = woutp.tile([128, H2], BF16, tag="wo")
                nc.sync.dma_start(w_t[:], wop_d[vt])
                po_t = pop.tile([128, T, BS], F32, tag="po")
                for hc in range(8):
                    nc.tensor.matmul(
                        po_t[:],
                        w_t[:, hc * 128 : (hc + 1) * 128],
                        hs_sb[:, hc],
                        start=(hc == 0),
                        stop=(hc == 7),
                    )
                sj = vt % STG
                if sj == 0:
                    stage_t = stagep.tile([128, STG, T, BS], BF16, tag="st")
                # bias-add + downcast on the (otherwise idle) vector engine
                nc.vector.tensor_scalar_add(
                    stage_t[:, sj], po_t[:], bout_sb[:, vt : vt + 1]
                )
                if sj == STG - 1 or vt == NVT - 1:
                    nv = sj + 1
                    dst = out_d[vt - sj : vt + 1].rearrange("j p c -> p j c")
                    src = stage_t[:, :nv].rearrange("p j t b -> p j (t b)")
                    nc.sync.dma_start(dst, src)

    nc.compile()
    return nc


def prep_host(features, captions, pad_idx, emb, W_ih, W_hh, b_ih, b_hh, W_out, b_out):
    """Host-side layout prep. Returns (shared dict, per-core list of dicts)."""
    from einops import rearrange

    features = np.asarray(features, dtype=np.float32)
    captions = np.asarray(captions).astype(np.int64)
    pad_idx = int(np.asarray(pad_idx))
    emb = np.asarray(emb, dtype=np.float32)
    W_ih = np.asarray(W_ih, dtype=np.float32)
    W_hh = np.asarray(W_hh, dtype=np.float32)
    b_ih = np.asarray(b_ih, dtype=np.float32)
    b_hh = np.asarray(b_hh, dtype=np.float32)
    W_out = np.asarray(W_out, dtype=np.float32)
    b_out = np.asarray(b_out, dtype=np.float32)

    embB = np.ascontiguousarray(emb.astype(bf16))
    wihT = np.ascontiguousarray(rearrange(W_ih, "g (kc p) -> kc p g", p=128).astype(bf16))
    whhT = np.ascontiguousarray(rearrange(W_hh, "g (hc p) -> hc p g", p=128).astype(fp8))
    bsum = rearrange(b_ih + b_hh, "(gc p) -> p gc", p=128).astype(np.float32)
    bsum = np.ascontiguousarray(np.repeat(bsum[:, :, None], BS, axis=2))

    W_out_p = np.zeros((VP, H2), np.float32)
    W_out_p[:V] = W_out
    wop = np.ascontiguousarray(
        rearrange(W_out_p, "(vt f) (hc p) -> vt p (hc f)", f=128, p=128).astype(bf16)
    )
    b_out_p = np.zeros((VP,), np.float32)
    b_out_p[:V] = b_out
    bout = np.ascontiguousarray(rearrange(b_out_p, "(vt p) -> p vt", p=128))

    shared = {"embB": embB, "wihT": wihT, "whhT": whhT, "bsum": bsum,
              "wop": wop, "bout": bout}

    per_core = []
    for c in range(NC_N):
        bsl = slice(c * BS, (c + 1) * BS)
        gidx = np.zeros((T, BS), np.int64)  # row r = t*BS + b
        gidx[0, :] = pad_idx
        gidx[1:, :] = captions[bsl, : T - 1].T
        idx = np.ascontiguousarray(
            gidx.reshape(3, 128).T.astype(np.int32)
        )  # [128, 3]: idx[p, r3] = gidx_flat[r3*128 + p]
        featT = np.ascontiguousarray(
            rearrange(features[bsl], "b (ec p) -> p ec b", p=128).astype(bf16)
        )
        per_core.append({"idx": idx, "featT": featT})
    return shared, per_core


_NC_CACHE = None


def kernel(**inputs) -> np.ndarray:
    global _NC_CACHE
    if _NC_CACHE is None:
        _NC_CACHE = build_nc()
    nc = _NC_CACHE

    shared, per_core = prep_host(**inputs)
    in_maps = [dict(shared, **pc) for pc in per_core]
    res = run_bass_kernel_spmd(nc, in_maps, core_ids=list(range(NC_N)))

    out = np.empty((B, V, T), np.float32)
    for c in range(NC_N):
        o = res.results[c]["out"]  # [NVT, 128, NCOL] bf16, col = t*BS + b
        o = np.asarray(o, dtype=np.float32).reshape(NVT, 128, T, BS)
        o = o.transpose(3, 0, 1, 2).reshape(BS, VP, T)
        out[c * BS : (c + 1) * BS] = o[:, :V, :]
    return out
